# revision 33
# baseline (speedup 1.0000x reference)
"""Trainium2 Bass kernel for nn_AttentionSimple (sparse_attention, 8 cores).

Algorithm: count-weighted vocab-space softmax — no per-token gathers.
Scores depend on s only through v = k[b, s], so group softmax terms by
vocabulary id:
    c[b, v]  = |{s : k[b, s] = v}|         (histogram of k, built on host)
    l[b, v]  = q[b] . embeddings[v]        (dense PE matmul)
    A        = c * exp(l)
    out[b]   = (sum_v A[b,v] * EW[v]) / (sum_v A[b,v])
    with EW  = embeddings @ W.T + b        (parameter prepacking, host)

Sharding: padded vocabulary (53248 = 416 chunks of 128) split across the
8 cores (52 chunks each); every core handles all 128 batch rows. Each
core returns partial numerators/denominators; the host unshard step sums
the 8 partials and divides (flash-style partial-softmax merge).

Per-core device pipeline:
  - embT2: two vocab chunks stacked on the matmul contraction dim, so
    one f32r matmul per chunk-pair computes 256 logit columns.
  - ACT: A = exp(ps) fused PSUM->SBUF per quad; DVE: A *= counts (u8).
  - PE:  acc[9, 512] += st9_quad.T @ A, st9 = [EW c0..c3 | ones].
  - 3 warm-up matmuls ramp the PE p-state while the first DMAs land.
  - All bulk DMAs ride one ordered Sync queue in exact consumption
    order; tiny qw/st DMAs go on the Scalar queue.
"""
import numpy as np

BATCH, SEQ, EMB, VOCAB, OUT = 128, 8192, 50, 50000, 2
N_CORES = 8
CSH = 52
NCHUNK = CSH * N_CORES
VPAD = NCHUNK * 128
VSH = CSH * 128
NPAIR = CSH // 2
NQUAD = NPAIR // 2
EPAD = 64
NQW = 2 * BATCH
GROUPS = [2, 2, 3, 3, 3]

_CACHE = {}


def _build_nc():
    from contextlib import ExitStack

    import concourse.mybir as mybir
    import concourse.tile as tile
    from concourse import bacc

    f32 = mybir.dt.float32
    f32r = mybir.dt.float32r
    nc = bacc.Bacc("TRN2", target_bir_lowering=False, debug=False,
                   num_devices=N_CORES)

    embT2_d = nc.dram_tensor("embT2", [128, NPAIR * 128], f32r,
                             kind="ExternalInput")
    qw_d = nc.dram_tensor("qw", [128, NQW], f32r, kind="ExternalInput")
    st_d = nc.dram_tensor("st", [128, NQUAD * 9], f32r,
                          kind="ExternalInput")
    ct_d = nc.dram_tensor("ct", [128, CSH * BATCH], mybir.dt.uint8,
                          kind="ExternalInput")
    o_d = nc.dram_tensor("o", [9, 4 * BATCH], f32, kind="ExternalOutput")

    with tile.TileContext(nc) as tc, ExitStack() as ctx:
        const_p = ctx.enter_context(tc.tile_pool(name="const", bufs=1))
        emb_p = ctx.enter_context(tc.tile_pool(name="embt", bufs=3))
        ct_p = ctx.enter_context(tc.tile_pool(name="ctp", bufs=3))
        le_p = ctx.enter_context(tc.tile_pool(name="le", bufs=6))
        ps_p = ctx.enter_context(tc.tile_pool(name="ps", bufs=6, space="PSUM"))
        acc_p = ctx.enter_context(tc.tile_pool(name="acc", bufs=1,
                                               space="PSUM"))
        wps_p = ctx.enter_context(tc.tile_pool(name="wps", bufs=1,
                                               space="PSUM"))
        fin_p = ctx.enter_context(tc.tile_pool(name="fin", bufs=1))

        wtile = const_p.tile([128, 512], f32r)
        nc.vector.memset(wtile[:].bitcast(f32), 0.0)
        wps = wps_p.tile([128, 512], f32)
        for _ in range(6):
            nc.tensor.matmul(wps[:], lhsT=wtile[:, 0:128], rhs=wtile[:],
                             start=True, stop=True)

        qw_sb = const_p.tile([128, NQW], f32r)
        nc.scalar.dma_start(qw_sb[:], qw_d.ap())
        st_sb = const_p.tile([128, NQUAD * 9], f32r)
        nc.scalar.dma_start(st_sb[:], st_d.ap())
        acc = acc_p.tile([9, 4 * BATCH], f32)

        quad0 = 0
        for gsz in GROUPS:
            et = emb_p.tile([128, 3 * 256], f32r, tag="et")
            nc.sync.dma_start(
                et[:, 0:gsz * 256],
                embT2_d.ap()[:, quad0 * 256:(quad0 + gsz) * 256])
            ctt = ct_p.tile([128, 3 * 512], mybir.dt.uint8, tag="ct")
            nc.sync.dma_start(
                ctt[:, 0:gsz * 512],
                ct_d.ap()[:, quad0 * 512:(quad0 + gsz) * 512])

            for lq in range(gsz):
                quad = quad0 + lq
                ps = ps_p.tile([128, 512], f32)
                for h in range(2):
                    nc.tensor.matmul(
                        ps[:, h * 256:(h + 1) * 256],
                        lhsT=et[:, lq * 256 + h * 128:lq * 256 + h * 128 + 128],
                        rhs=qw_sb[:],
                        start=True, stop=True,
                    )
                le = le_p.tile([128, 512], f32r)
                nc.scalar.activation(le[:], ps[:],
                                     mybir.ActivationFunctionType.Exp)
                nc.vector.tensor_mul(
                    le[:], le[:], ctt[:, lq * 512:(lq + 1) * 512])
                nc.tensor.matmul(
                    acc[:],
                    lhsT=st_sb[:, quad * 9:(quad + 1) * 9],
                    rhs=le[:],
                    start=(quad == 0), stop=(quad == NQUAD - 1),
                    skip_group_check=True,
                )
            quad0 += gsz

        osb = fin_p.tile([9, 4 * BATCH], f32)
        nc.vector.tensor_copy(osb[:], acc[:])
        nc.sync.dma_start(o_d.ap(), osb[:])

    nc.finalize()
    return nc


def _prep_inputs(q, k, embeddings, W, b):
    q = np.ascontiguousarray(q, dtype=np.float32)
    emb = np.ascontiguousarray(embeddings, dtype=np.float32)
    W = np.ascontiguousarray(W, dtype=np.float32)
    b = np.ascontiguousarray(b, dtype=np.float32)
    k = np.asarray(k)

    embT = np.zeros((EMB, VPAD), np.float32)
    embT[:, :VOCAB] = emb.T

    qw = np.zeros((128, NQW), np.float32)
    qw[:EMB, 0:BATCH] = q.T
    qw[EPAD:EPAD + EMB, BATCH:2 * BATCH] = q.T

    EWp = np.zeros((VPAD, OUT), np.float32)
    EWp[:VOCAB] = emb @ W.T + b[None, :]

    flat = (np.arange(BATCH, dtype=np.int64)[:, None] * VPAD
            + k.astype(np.int64)).ravel()
    C = np.bincount(flat, minlength=BATCH * VPAD).reshape(BATCH, VPAD)
    assert C.max() <= 255
    C = C.astype(np.float32)

    in_maps = []
    for core in range(N_CORES):
        v0 = core * VSH
        blocks = embT[:, v0:v0 + VSH].reshape(EMB, CSH, 128)
        e2 = np.zeros((128, NPAIR, 128), np.float32)
        e2[:EMB] = blocks[:, 0::2, :]
        e2[EPAD:EPAD + EMB] = blocks[:, 1::2, :]
        e2 = np.ascontiguousarray(e2.reshape(128, NPAIR * 128))

        ew_blocks = EWp[v0:v0 + VSH].reshape(CSH, 128, OUT)
        st = np.zeros((128, NQUAD, 9), np.float32)
        for j in range(4):
            st[:, :, 2 * j:2 * j + 2] = (
                ew_blocks.reshape(NQUAD, 4, 128, OUT)[:, j]
                .transpose(1, 0, 2))
        st[:, :, 8] = 1.0
        st = np.ascontiguousarray(st.reshape(128, NQUAD * 9))

        ct = np.ascontiguousarray(
            C[:, v0:v0 + VSH].reshape(BATCH, CSH, 128)
            .transpose(2, 1, 0).reshape(128, CSH * BATCH)
            .astype(np.uint8))
        in_maps.append({"embT2": e2, "qw": qw, "st": st, "ct": ct})
    return in_maps


def _run_device(in_maps, **kwargs):
    from concourse.bass_utils import run_bass_kernel_spmd

    if "nc" not in _CACHE:
        _CACHE["nc"] = _build_nc()
    return run_bass_kernel_spmd(_CACHE["nc"], in_maps,
                                core_ids=list(range(N_CORES)), **kwargs)


def _unshard(res):
    P = np.zeros((9, 4 * BATCH), np.float64)
    for i in range(N_CORES):
        P += res.results[i]["o"].astype(np.float64)
    numer = np.zeros((OUT, BATCH), np.float64)
    denom = np.zeros(BATCH, np.float64)
    for j in range(4):
        numer += P[2 * j:2 * j + 2, j * BATCH:(j + 1) * BATCH]
        denom += P[8, j * BATCH:(j + 1) * BATCH]
    out = (numer / denom[None, :]).T
    return np.ascontiguousarray(out, dtype=np.float32)


def kernel(q, k, embeddings, W, b, **_unused):
    in_maps = _prep_inputs(q, k, embeddings, W, b)
    res = _run_device(in_maps)
    return _unshard(res)


# revision 34
# speedup vs baseline: 1.0174x; 1.0174x over previous
"""Trainium2 Bass kernel for nn_AttentionSimple (sparse_attention, 8 cores).

Algorithm: count-weighted vocab-space softmax — no per-token gathers.
Scores depend on s only through v = k[b, s], so group softmax terms by
vocabulary id:
    c[b, v]  = |{s : k[b, s] = v}|         (histogram of k, built on host)
    l[b, v]  = q[b] . embeddings[v]        (dense PE matmul)
    A        = c * exp(l)
    out[b]   = (sum_v A[b,v] * EW[v]) / (sum_v A[b,v])
    with EW  = embeddings @ W.T + b        (parameter prepacking, host)

Sharding: padded vocabulary (53248 = 416 chunks of 128) split across the
8 cores (52 chunks each); every core handles all 128 batch rows. Each
core returns partial numerators/denominators; the host unshard step sums
the 8 partials and divides (flash-style partial-softmax merge).

Per-core device pipeline:
  - embT2: two vocab chunks stacked on the matmul contraction dim, so
    one f32r matmul per chunk-pair computes 256 logit columns.
  - ACT: A = exp(ps) fused PSUM->SBUF per quad; DVE: A *= counts (u8).
  - PE:  acc[9, 512] += st9_quad.T @ A, st9 = [EW c0..c3 | ones].
  - 5 warm-up matmuls ramp the PE p-state while the first DMAs land
    (sized to end right as the qw DMA semaphore fires; any idle gap
    resets the 0.65->2.4GHz ramp, so the dovetail matters).
  - All bulk DMAs ride one ordered Sync queue in exact consumption
    order; tiny qw/st DMAs go on the Scalar queue.
"""
import numpy as np

BATCH, SEQ, EMB, VOCAB, OUT = 128, 8192, 50, 50000, 2
N_CORES = 8
CSH = 52
NCHUNK = CSH * N_CORES
VPAD = NCHUNK * 128
VSH = CSH * 128
NPAIR = CSH // 2
NQUAD = NPAIR // 2
EPAD = 64
NQW = 2 * BATCH
GROUPS = [2, 2, 3, 3, 3]

_CACHE = {}


def _build_nc():
    from contextlib import ExitStack

    import concourse.mybir as mybir
    import concourse.tile as tile
    from concourse import bacc

    f32 = mybir.dt.float32
    f32r = mybir.dt.float32r
    nc = bacc.Bacc("TRN2", target_bir_lowering=False, debug=False,
                   num_devices=N_CORES)

    embT2_d = nc.dram_tensor("embT2", [128, NPAIR * 128], f32r,
                             kind="ExternalInput")
    qw_d = nc.dram_tensor("qw", [128, NQW], f32r, kind="ExternalInput")
    st_d = nc.dram_tensor("st", [128, NQUAD * 9], f32r,
                          kind="ExternalInput")
    ct_d = nc.dram_tensor("ct", [128, CSH * BATCH], mybir.dt.uint8,
                          kind="ExternalInput")
    o_d = nc.dram_tensor("o", [9, 4 * BATCH], f32, kind="ExternalOutput")

    with tile.TileContext(nc) as tc, ExitStack() as ctx:
        const_p = ctx.enter_context(tc.tile_pool(name="const", bufs=1))
        emb_p = ctx.enter_context(tc.tile_pool(name="embt", bufs=3))
        ct_p = ctx.enter_context(tc.tile_pool(name="ctp", bufs=3))
        le_p = ctx.enter_context(tc.tile_pool(name="le", bufs=6))
        ps_p = ctx.enter_context(tc.tile_pool(name="ps", bufs=6, space="PSUM"))
        acc_p = ctx.enter_context(tc.tile_pool(name="acc", bufs=1,
                                               space="PSUM"))
        wps_p = ctx.enter_context(tc.tile_pool(name="wps", bufs=1,
                                               space="PSUM"))
        fin_p = ctx.enter_context(tc.tile_pool(name="fin", bufs=1))

        wtile = const_p.tile([128, 512], f32r)
        nc.vector.memset(wtile[:].bitcast(f32), 0.0)
        wps = wps_p.tile([128, 512], f32)
        for _ in range(5):
            nc.tensor.matmul(wps[:], lhsT=wtile[:, 0:128], rhs=wtile[:],
                             start=True, stop=True)

        qw_sb = const_p.tile([128, NQW], f32r)
        nc.scalar.dma_start(qw_sb[:], qw_d.ap())
        st_sb = const_p.tile([128, NQUAD * 9], f32r)
        nc.scalar.dma_start(st_sb[:], st_d.ap())
        acc = acc_p.tile([9, 4 * BATCH], f32)

        quad0 = 0
        for gsz in GROUPS:
            et = emb_p.tile([128, 3 * 256], f32r, tag="et")
            nc.sync.dma_start(
                et[:, 0:gsz * 256],
                embT2_d.ap()[:, quad0 * 256:(quad0 + gsz) * 256])
            ctt = ct_p.tile([128, 3 * 512], mybir.dt.uint8, tag="ct")
            nc.sync.dma_start(
                ctt[:, 0:gsz * 512],
                ct_d.ap()[:, quad0 * 512:(quad0 + gsz) * 512])

            for lq in range(gsz):
                quad = quad0 + lq
                ps = ps_p.tile([128, 512], f32)
                for h in range(2):
                    nc.tensor.matmul(
                        ps[:, h * 256:(h + 1) * 256],
                        lhsT=et[:, lq * 256 + h * 128:lq * 256 + h * 128 + 128],
                        rhs=qw_sb[:],
                        start=True, stop=True,
                    )
                le = le_p.tile([128, 512], f32r)
                nc.scalar.activation(le[:], ps[:],
                                     mybir.ActivationFunctionType.Exp)
                nc.vector.tensor_mul(
                    le[:], le[:], ctt[:, lq * 512:(lq + 1) * 512])
                nc.tensor.matmul(
                    acc[:],
                    lhsT=st_sb[:, quad * 9:(quad + 1) * 9],
                    rhs=le[:],
                    start=(quad == 0), stop=(quad == NQUAD - 1),
                    skip_group_check=True,
                )
            quad0 += gsz

        osb = fin_p.tile([9, 4 * BATCH], f32)
        nc.vector.tensor_copy(osb[:], acc[:])
        nc.sync.dma_start(o_d.ap(), osb[:])

    nc.finalize()
    return nc


def _prep_inputs(q, k, embeddings, W, b):
    q = np.ascontiguousarray(q, dtype=np.float32)
    emb = np.ascontiguousarray(embeddings, dtype=np.float32)
    W = np.ascontiguousarray(W, dtype=np.float32)
    b = np.ascontiguousarray(b, dtype=np.float32)
    k = np.asarray(k)

    embT = np.zeros((EMB, VPAD), np.float32)
    embT[:, :VOCAB] = emb.T

    qw = np.zeros((128, NQW), np.float32)
    qw[:EMB, 0:BATCH] = q.T
    qw[EPAD:EPAD + EMB, BATCH:2 * BATCH] = q.T

    EWp = np.zeros((VPAD, OUT), np.float32)
    EWp[:VOCAB] = emb @ W.T + b[None, :]

    flat = (np.arange(BATCH, dtype=np.int64)[:, None] * VPAD
            + k.astype(np.int64)).ravel()
    C = np.bincount(flat, minlength=BATCH * VPAD).reshape(BATCH, VPAD)
    assert C.max() <= 255
    C = C.astype(np.float32)

    in_maps = []
    for core in range(N_CORES):
        v0 = core * VSH
        blocks = embT[:, v0:v0 + VSH].reshape(EMB, CSH, 128)
        e2 = np.zeros((128, NPAIR, 128), np.float32)
        e2[:EMB] = blocks[:, 0::2, :]
        e2[EPAD:EPAD + EMB] = blocks[:, 1::2, :]
        e2 = np.ascontiguousarray(e2.reshape(128, NPAIR * 128))

        ew_blocks = EWp[v0:v0 + VSH].reshape(CSH, 128, OUT)
        st = np.zeros((128, NQUAD, 9), np.float32)
        for j in range(4):
            st[:, :, 2 * j:2 * j + 2] = (
                ew_blocks.reshape(NQUAD, 4, 128, OUT)[:, j]
                .transpose(1, 0, 2))
        st[:, :, 8] = 1.0
        st = np.ascontiguousarray(st.reshape(128, NQUAD * 9))

        ct = np.ascontiguousarray(
            C[:, v0:v0 + VSH].reshape(BATCH, CSH, 128)
            .transpose(2, 1, 0).reshape(128, CSH * BATCH)
            .astype(np.uint8))
        in_maps.append({"embT2": e2, "qw": qw, "st": st, "ct": ct})
    return in_maps


def _run_device(in_maps, **kwargs):
    from concourse.bass_utils import run_bass_kernel_spmd

    if "nc" not in _CACHE:
        _CACHE["nc"] = _build_nc()
    return run_bass_kernel_spmd(_CACHE["nc"], in_maps,
                                core_ids=list(range(N_CORES)), **kwargs)


def _unshard(res):
    P = np.zeros((9, 4 * BATCH), np.float64)
    for i in range(N_CORES):
        P += res.results[i]["o"].astype(np.float64)
    numer = np.zeros((OUT, BATCH), np.float64)
    denom = np.zeros(BATCH, np.float64)
    for j in range(4):
        numer += P[2 * j:2 * j + 2, j * BATCH:(j + 1) * BATCH]
        denom += P[8, j * BATCH:(j + 1) * BATCH]
    out = (numer / denom[None, :]).T
    return np.ascontiguousarray(out, dtype=np.float32)


def kernel(q, k, embeddings, W, b, **_unused):
    in_maps = _prep_inputs(q, k, embeddings, W, b)
    res = _run_device(in_maps)
    return _unshard(res)
